# revision 23
# baseline (speedup 1.0000x reference)
"""trn2 Bass kernel for the nn_RNN problem (B=64, T=1024, I=H=O=512, fp32).

Data-parallel over batch: 8 cores x 8 batch rows each. Per core:
  phase A: xw^T = (X @ w_xh + bias_h)^T  -- GEMM kept on-chip, transposed
  phase B: 1024 sequential steps h = tanh(xw_t + h @ w_hh), carried as h^T
  phase C: y = h @ w_out.T + b_out

Every matmul / add / tanh replicates the XLA-Neuron fp32 lowering of the
reference ops on this hardware, so the result is bit-identical to running
the reference in jax on these devices:
  - big-M GEMM (phase A): stationary = x^T m-tile (fp32 LOW/HIGH split on X),
    moving = w_xh k-tile [128,512], k-tiles ascending into one PSUM group,
    output in natural layout, then exact PE transpose + single-rounding bias
    add in transposed space.
  - small-M matmuls (phases B/C): stationary = weight tile [128k,128n]
    (LOW/HIGH split on the weight), moving = data^T [128k,m], k ascending.
  - tanh: ScalarE ACT TANH, scale=1, bias=0 (same LUT as XLA).
  - adds: single fp32 roundings in the reference association order.
"""

import numpy as np
from contextlib import ExitStack

import concourse.bass as bass
import concourse.tile as tile
from concourse import bacc, mybir, bass_utils
from concourse.masks import make_identity

B, T, I, H, O = 64, 1024, 512, 512, 512
NCORES = 8
BS = B // NCORES
KT = 4   # 512/128 contraction tiles
NB = 4   # 512/128 output blocks
TB = 64  # timesteps per phase-A block
NTB = T // TB
F32 = mybir.dt.float32


def _build(nc, order="nb", split_tanh=True, rec_bufs=3, tail_inline=False, y_tp=True,
           repeats=1):
    X_d = nc.dram_tensor("X", (BS, T, I), F32, kind="ExternalInput").ap()
    wxh_d = nc.dram_tensor("w_xh", (I, H), F32, kind="ExternalInput").ap()
    whh_d = nc.dram_tensor("w_hh", (H, H), F32, kind="ExternalInput").ap()
    bh_d = nc.dram_tensor("bias_h", (H,), F32, kind="ExternalInput").ap()
    wout_d = nc.dram_tensor("w_out", (O, H), F32, kind="ExternalInput").ap()
    bout_d = nc.dram_tensor("b_out", (O,), F32, kind="ExternalInput").ap()
    y_d = nc.dram_tensor("y", (BS, O), F32, kind="ExternalOutput").ap()

    M = BS * TB  # moving rows per phase-A t-block
    BPM = 128 // TB  # batches per 128-row m-tile

    with tile.TileContext(nc) as tc, ExitStack() as ctx:
        const = ctx.enter_context(tc.tile_pool(name="const", bufs=1))
        big = ctx.enter_context(tc.tile_pool(name="big", bufs=1))

        ident = const.tile([128, 128], F32)
        make_identity(nc, ident[:])

        wxh_sb = const.tile([128, KT, H], F32)    # [k, kt, n] natural rows
        whh_sb = const.tile([128, KT, H], F32)
        woutT_sb = const.tile([128, KT, O], F32)  # [in_k, kt, o]
        bh_sb = const.tile([128, NB], F32)        # biases, transposed space
        bout_sb = const.tile([128, NB], F32)
        for kt in range(KT):
            nc.sync.dma_start(out=wxh_sb[:, kt, :], in_=wxh_d[kt * 128:(kt + 1) * 128, :])
            nc.sync.dma_start(out=whh_sb[:, kt, :], in_=whh_d[kt * 128:(kt + 1) * 128, :])
        nc.sync.dma_start(out=bh_sb[:], in_=bh_d.rearrange("(nb p) -> p nb", p=128))
        nc.sync.dma_start(out=bout_sb[:], in_=bout_d.rearrange("(nb p) -> p nb", p=128))

        # ---------------- Phase A ----------------
        xwT = big.tile([128, NB, NTB, BS, TB], F32)

        with ExitStack() as actx:
            wt_ps = actx.enter_context(tc.tile_pool(name="wt_ps", bufs=2, space="PSUM"))
            wtmp_pool = actx.enter_context(tc.tile_pool(name="wtmp", bufs=1))
            wout_nat = wtmp_pool.tile([128, NB, H], F32)  # [o, ot, in] natural rows
            for ot in range(NB):
                nc.sync.dma_start(out=wout_nat[:, ot, :], in_=wout_d[ot * 128:(ot + 1) * 128, :])
            for kt in range(KT):          # "in" blocks
                tp = wt_ps.tile([128, H], F32)
                for ot in range(NB):      # "out" blocks
                    nc.tensor.transpose(tp[:, ot * 128:(ot + 1) * 128],
                                        wout_nat[:, ot, kt * 128:(kt + 1) * 128], ident[:])
                nc.vector.tensor_copy(woutT_sb[:, kt, :], tp[:])

            xin_pool = actx.enter_context(tc.tile_pool(name="xin", bufs=1))
            xt_pool = actx.enter_context(tc.tile_pool(name="xt", bufs=2))
            xwn_pool = actx.enter_context(tc.tile_pool(name="xwn", bufs=3))
            tp_ps = actx.enter_context(tc.tile_pool(name="tp_ps", bufs=2, space="PSUM"))
            gemm_ps = actx.enter_context(tc.tile_pool(name="gemm_ps", bufs=2, space="PSUM"))

            for tb in range(NTB):
                xin = xin_pool.tile([TB, BS, I], F32)
                for b in range(BS):
                    nc.sync.dma_start(out=xin[:, b, :], in_=X_d[b, tb * TB:(tb + 1) * TB, :])
                xt = xt_pool.tile([128, KT, M], F32)
                for kt in range(KT):
                    tp = tp_ps.tile([128, M], F32, tag="tp")
                    for b in range(BS):
                        nc.tensor.transpose(tp[:, b * TB:(b + 1) * TB],
                                            xin[:, b, kt * 128:(kt + 1) * 128], ident[:TB, :TB])
                    nc.vector.tensor_copy(xt[:, kt, :], tp[:])
                for mt in range(M // 128):
                    acc = gemm_ps.tile([128, H], F32)
                    for kt in range(KT):
                        nc.tensor.matmul(acc[:], lhsT=xt[:, kt, mt * 128:(mt + 1) * 128],
                                         rhs=wxh_sb[:, kt, :], start=(kt == 0), stop=(kt == KT - 1))
                    xw_nat = xwn_pool.tile([128, H], F32)
                    nc.vector.tensor_copy(xw_nat[:], acc[:])
                    for nb in range(NB):
                        tp2 = tp_ps.tile([128, 128], F32, tag="tp2")
                        nc.tensor.transpose(tp2[:], xw_nat[:, nb * 128:(nb + 1) * 128], ident[:])
                        nc.vector.tensor_scalar_add(
                            xwT[:, nb, tb, mt * BPM:(mt + 1) * BPM, :], tp2[:], bh_sb[:, nb:nb + 1])

        # ---------------- Phase B ----------------
        h_pool = ctx.enter_context(tc.tile_pool(name="h", bufs=3))
        pre_pool = ctx.enter_context(tc.tile_pool(name="pre", bufs=3))
        rec_ps = ctx.enter_context(tc.tile_pool(name="rec_ps", bufs=rec_bufs, space="PSUM"))

        for _rep in range(repeats):
          hT = h_pool.tile([128, KT, BS], F32, tag="hT")
          nc.vector.memset(hT[:], 0.0)
          for t in range(T):
            tb, tl = t // TB, t % TB
            acc = rec_ps.tile([128, NB, BS], F32, tag="rec")
            pre = pre_pool.tile([128, NB, BS], F32, tag="pre")
            hT_new = h_pool.tile([128, KT, BS], F32, tag="hT")

            def tail(nb):
                nc.vector.tensor_add(pre[:, nb, :], xwT[:, nb, tb, :, tl], acc[:, nb, :])
                if split_tanh:
                    nc.scalar.activation(hT_new[:, nb, :], pre[:, nb, :],
                                         mybir.ActivationFunctionType.Tanh)

            if order == "kt":
                # kt-major: each PSUM group still accumulates kt-ascending
                # (identical bits), but step t+1's kt=0 matmuls only need the
                # kt=0 slice of hT, letting tanh overlap the next step.
                for kt in range(KT):
                    for nb in range(NB):
                        nc.tensor.matmul(acc[:, nb, :], lhsT=whh_sb[:, kt, nb * 128:(nb + 1) * 128],
                                         rhs=hT[:, kt, :], start=(kt == 0), stop=(kt == KT - 1))
                        if tail_inline and kt == KT - 1:
                            tail(nb)
            else:
                for nb in range(NB):
                    for kt in range(KT):
                        nc.tensor.matmul(acc[:, nb, :], lhsT=whh_sb[:, kt, nb * 128:(nb + 1) * 128],
                                         rhs=hT[:, kt, :], start=(kt == 0), stop=(kt == KT - 1))
                    if tail_inline:
                        tail(nb)
            if not tail_inline:
                for nb in range(NB):
                    tail(nb)
            if not split_tanh:
                nc.scalar.activation(hT_new[:], pre[:], mybir.ActivationFunctionType.Tanh)
            hT = hT_new

        # ---------------- Phase C ----------------
        acc = rec_ps.tile([128, NB, BS], F32, tag="rec")
        for nb in range(NB):
            for kt in range(KT):
                nc.tensor.matmul(acc[:, nb, :], lhsT=woutT_sb[:, kt, nb * 128:(nb + 1) * 128],
                                 rhs=hT[:, kt, :], start=(kt == 0), stop=(kt == KT - 1))
        yT = pre_pool.tile([128, NB, BS], F32, tag="pre")
        for nb in range(NB):
            nc.vector.tensor_scalar_add(yT[:, nb, :], acc[:, nb, :], bout_sb[:, nb:nb + 1])
        if y_tp:
            # exact transpose back to natural [BS, O] for one contiguous DMA
            y_ps = rec_ps.tile([BS, O], F32, tag="yps")
            for nb in range(NB):
                nc.tensor.transpose(y_ps[:, nb * 128:(nb + 1) * 128], yT[:, nb, :], ident[:])
            y_sb = pre_pool.tile([BS, O], F32, tag="ysb")
            nc.vector.tensor_copy(y_sb[:], y_ps[:])
            nc.sync.dma_start(out=y_d[:], in_=y_sb[:])
        else:
            for nb in range(NB):
                nc.sync.dma_start(out=y_d.transpose([1, 0])[nb * 128:(nb + 1) * 128, :],
                                  in_=yT[:, nb, :])


_CACHE = {}


def get_nc(**build_kw):
    key = ("nc",) + tuple(sorted(build_kw.items()))
    if key not in _CACHE:
        nc = bacc.Bacc("TRN2", target_bir_lowering=False, debug=False, num_devices=NCORES)
        _build(nc, **build_kw)
        nc.compile()
        _CACHE[key] = nc
    return _CACHE[key]


def get_runner(**build_kw):
    """One-time lowering of the bass module to a cached jitted PJRT callable.

    run_bass_kernel_spmd rebuilds + re-jits the wrapper on every call (the
    jit cache keys on a fresh closure), which costs seconds per invocation.
    This replicates its multi-core tail once and reuses the compiled
    executable for repeat calls.
    """
    rkey = ("runner",) + tuple(sorted(build_kw.items()))
    if rkey in _CACHE:
        return _CACHE[rkey]

    import jax
    import jax.numpy as jnp
    from jax.sharding import Mesh, PartitionSpec
    from jax.experimental.shard_map import shard_map
    from concourse import bass2jax, mybir as _mybir

    nc = get_nc(**build_kw)
    bass2jax.install_neuronx_cc_hook()

    partition_name = nc.partition_id_tensor.name if nc.partition_id_tensor else None
    in_names, out_names, out_avals, zero_outs = [], [], [], []
    for alloc in nc.m.functions[0].allocations:
        if not isinstance(alloc, _mybir.MemoryLocationSet):
            continue
        name = alloc.memorylocations[0].name
        if alloc.kind == "ExternalInput":
            if name != partition_name:
                in_names.append(name)
        elif alloc.kind == "ExternalOutput":
            shape = tuple(alloc.tensor_shape)
            dt = _mybir.dt.np(alloc.dtype)
            out_names.append(name)
            out_avals.append(jax.core.ShapedArray(shape, dt))
            zero_outs.append(np.zeros(shape, dt))
    n_params, n_outs = len(in_names), len(out_names)
    param_names = list(in_names)
    in_names.extend(out_names)
    if partition_name is not None:
        in_names.append(partition_name)
    donate = tuple(range(n_params, n_params + n_outs))

    def _body(*args):
        operands = list(args)
        if partition_name is not None:
            operands.append(bass2jax.partition_id_tensor())
        outs = bass2jax._bass_exec_p.bind(
            *operands,
            out_avals=tuple(out_avals),
            in_names=tuple(in_names),
            out_names=tuple(out_names),
            lowering_input_output_aliases=(),
            sim_require_finite=True,
            sim_require_nnan=True,
            nc=nc,
        )
        return tuple(outs)

    devices = jax.devices()[:NCORES]
    mesh = Mesh(np.asarray(devices), ("core",))
    sharded = jax.jit(
        shard_map(_body, mesh=mesh,
                  in_specs=(PartitionSpec("core"),) * (n_params + n_outs),
                  out_specs=(PartitionSpec("core"),) * n_outs,
                  check_rep=False),
        donate_argnums=donate, keep_unused=True,
    )

    def run(in_maps):
        concat_in = [
            np.concatenate([np.asarray(in_maps[c][name]) for c in range(NCORES)], axis=0)
            for name in param_names
        ]
        concat_zeros = [np.zeros((NCORES * z.shape[0], *z.shape[1:]), z.dtype) for z in zero_outs]
        out_arrs = sharded(*concat_in, *concat_zeros)
        return [
            {name: np.asarray(out_arrs[i]).reshape(NCORES, *out_avals[i].shape)[c]
             for i, name in enumerate(out_names)}
            for c in range(NCORES)
        ]

    _CACHE[rkey] = {
        "run": run, "sharded": sharded, "mesh": mesh, "param_names": param_names,
        "out_names": out_names, "out_avals": out_avals, "zero_outs": zero_outs,
    }
    return _CACHE[rkey]


def time_kernel(X, w_xh, w_hh, bias_h, w_out, b_out, iters=10):
    """Best-of-N wall time of the compiled executable with device-resident
    inputs (excludes host->device transfer of X and jit overhead)."""
    import time as _time
    import jax
    from jax.sharding import NamedSharding, PartitionSpec

    r = get_runner()
    in_maps = make_in_maps(X, w_xh, w_hh, bias_h, w_out, b_out)
    concat_in = [
        np.concatenate([np.asarray(in_maps[c][name]) for c in range(NCORES)], axis=0)
        for name in r["param_names"]
    ]
    sh = NamedSharding(r["mesh"], PartitionSpec("core"))
    dev_in = [jax.device_put(a, sh) for a in concat_in]
    times = []
    for _ in range(iters):
        zeros = [np.zeros((NCORES * z.shape[0], *z.shape[1:]), z.dtype) for z in r["zero_outs"]]
        t0 = _time.perf_counter()
        out = r["sharded"](*dev_in, *zeros)
        jax.block_until_ready(out)
        times.append(_time.perf_counter() - t0)
    return min(times), times


def make_in_maps(X, w_xh, w_hh, bias_h, w_out, b_out):
    f32c = lambda a: np.ascontiguousarray(np.asarray(a, dtype=np.float32))
    X, w_xh, w_hh, bias_h, w_out, b_out = map(f32c, (X, w_xh, w_hh, bias_h, w_out, b_out))
    return [
        {"X": np.ascontiguousarray(X[c * BS:(c + 1) * BS]), "w_xh": w_xh, "w_hh": w_hh,
         "bias_h": bias_h, "w_out": w_out, "b_out": b_out}
        for c in range(NCORES)
    ]


def kernel(X, w_xh, w_hh, bias_h, w_out, b_out):
    r = get_runner()
    arrs = {"X": X, "w_xh": w_xh, "w_hh": w_hh, "bias_h": bias_h,
            "w_out": w_out, "b_out": b_out}

    import jax
    import jax.numpy as jnp
    is_jax = isinstance(X, jax.Array)

    concat = {}
    if is_jax:
        # keep everything device-side; X [64,1024,512] is already the
        # 8-shard concat layout, weights replicate via tile
        concat["X"] = jnp.asarray(X, jnp.float32)
        for name in ("w_xh", "w_hh", "bias_h", "w_out", "b_out"):
            a = jnp.asarray(arrs[name], jnp.float32)
            concat[name] = jnp.tile(a, (NCORES,) + (1,) * (a.ndim - 1))
    else:
        concat["X"] = np.asarray(X, np.float32)
        for name in ("w_xh", "w_hh", "bias_h", "w_out", "b_out"):
            a = np.asarray(arrs[name], np.float32)
            concat[name] = np.tile(a, (NCORES,) + (1,) * (a.ndim - 1))

    concat_in = [concat[n] for n in r["param_names"]]
    zeros = [np.zeros((NCORES * z.shape[0], *z.shape[1:]), z.dtype) for z in r["zero_outs"]]
    out_arrs = r["sharded"](*concat_in, *zeros)
    y = np.asarray(out_arrs[r["out_names"].index("y")])  # [64, 512], shards in batch order
    return np.ascontiguousarray(y.astype(np.float32))
